# revision 87
# baseline (speedup 1.0000x reference)
"""Trainium2 Bass kernel for nn_AV_MiltiHeadAttention.

Strategy: data-parallel over B across 8 NeuronCores (1 batch element per core).
Per-core everything is kept in a "transposed" (feature-on-partitions) layout so
no on-chip transposes are needed:

  audiaT/lipT/W*T are marshalled on host (transpose + bf16 cast = input
  layout prep for the SPMD shards), all matmul contractions then have their
  contraction dim on SBUF partitions naturally.

  S.T[m,n] = sum_d kTz[d,m] qT[d,n]         (zero-padded per head: K=128, so
                                             every matmul shares one PE tiling
                                             mode -> no array-drain switches)
  E = exp(S.T * scale)                       (no max subtraction; |S*scale|<2)
  raw[j,n] = sum_m [k_h | 1][m,j] E[m,n]    -> rows 0..63 = attn_qk.T (unnorm),
                                               row 64 = softmax denominator r
  1/r, 1/s chains on [1,N] rows (DVE recip; partition-0 constraint), placed
  at rows 0/64 of zeroed [65,N] tiles; K=65 selector matmuls broadcast them
  across all 128 partitions in PSUM (65 rounds to row-size 128: no mode
  switch), so each normalize is a single [128,N] tensor_tensor.
  y = attn_qk.T * lip_v.T ; expY = exp(y)
  s = colsum_d(expY) via [128,65] selector matmul (softmax over head_dim)
  z.T = attn_qk.T * expY * (1/s)
  av_attn = sigmoid(2 * qs.T@qs / temp)      (SE layer)
  Wc = W_proj.T @ av_attn ; row = b_proj @ av_attn   (proj/out fusion)
  out = z @ Wc + row                          (single fused output GEMM; bias
                                             and cb<3 partials accumulate
                                             early, only cb=3 + add + DMA
                                             remain after the last z lands)

All matmuls bf16 x bf16 -> fp32 PSUM. Softmax normalization in fp32.
"""

import os
import numpy as np
import ml_dtypes

DEBUG_DUMP = os.environ.get("KDBG", "0") == "1"

B, N, C = 8, 1024, 512
H, HD = 8, 64
CB = C // 128          # 4 chunks of the feature dim
MB = N // 128          # 8 chunks of the token dim
NH = N // 512          # 2 halves of the token dim (matmul free dim = 512)
SCALE = HD ** -0.5
TEMP = C ** 0.5

_CACHED = {}


def build_nc():
    import concourse.bass as bass
    import concourse.tile as tile
    import concourse.mybir as mybir
    from concourse import bacc
    from contextlib import ExitStack

    f32 = mybir.dt.float32
    bf16 = mybir.dt.bfloat16
    AF = mybir.ActivationFunctionType
    MUL = mybir.AluOpType.mult
    ADD = mybir.AluOpType.add

    nc = bacc.Bacc("TRN2", target_bir_lowering=False, debug=False, num_devices=B)

    d_audiaT = nc.dram_tensor("audiaT", [C, N], bf16, kind="ExternalInput")
    d_lipT = nc.dram_tensor("lipT", [C, N], bf16, kind="ExternalInput")
    d_WqkT = nc.dram_tensor("WqkT", [C, 2 * C], bf16, kind="ExternalInput")
    d_WlipT = nc.dram_tensor("WlipT", [C, C], bf16, kind="ExternalInput")
    d_WseT = nc.dram_tensor("WseT", [C, C], bf16, kind="ExternalInput")
    d_WprojN = nc.dram_tensor("WprojN", [C, C], bf16, kind="ExternalInput")
    d_bprojP = nc.dram_tensor("bprojP", [128, CB, HD + 1], bf16, kind="ExternalInput")
    d_sel = nc.dram_tensor("sel", [128, HD + 1], bf16, kind="ExternalInput")
    d_selC = nc.dram_tensor("selC", [HD + 1, 128], bf16, kind="ExternalInput")
    d_onesK = nc.dram_tensor("onesK", [HD + 1, 128], bf16, kind="ExternalInput")
    d_out = nc.dram_tensor("out", [N, C], bf16, kind="ExternalOutput")
    dbg = {}
    if DEBUG_DUMP:
        for nm, shp, dt in [
            ("dbg_qT", [128, CB, N], bf16), ("dbg_kTz", [128, CB, 2, N], bf16),
            ("dbg_kaug", [128, MB, H * (HD + 1)], bf16),
            ("dbg_lipv", [128, CB, N], bf16), ("dbg_qs", [128, MB, C], bf16),
            ("dbg_E0", [128, MB, 2 * N], bf16),
            ("dbg_rawsb0", [HD, N], bf16), ("dbg_attn0", [128, N], bf16),
            ("dbg_r00", [1, N], f32),
            ("dbg_y0", [128, N], bf16), ("dbg_expY0", [128, N], bf16),
            ("dbg_sblo0", [HD, N], f32), ("dbg_sbhi0", [HD, N], f32),
            ("dbg_zT", [128, CB, N], bf16), ("dbg_avattn", [128, CB, C], bf16),
            ("dbg_wc", [128, CB, C], bf16),
        ]:
            dbg[nm] = nc.dram_tensor(nm, shp, dt, kind="ExternalOutput")

    with tile.TileContext(nc) as tc, ExitStack() as ctx:
        persist = ctx.enter_context(tc.tile_pool(name="persist", bufs=1))
        psA = ctx.enter_context(tc.tile_pool(name="psA", bufs=2, space="PSUM"))
        psB = ctx.enter_context(tc.tile_pool(name="psB", bufs=2, space="PSUM"))

        # ---- persistent SBUF tensors ----
        qT = persist.tile([128, CB, N], bf16, tag="qT")        # q.T  [(h,d), n]
        # k.T zero-padded per head so the S matmuls run K=128 (no PE
        # tiling-mode switches): chunk 0 = [k_h_lo; 0], chunk 1 = [0; k_h_hi]
        kTz = persist.tile([128, CB, 2, N], bf16, tag="kTz")
        nc.vector.memset(kTz[64:128, :, 0, :], 0.0)
        nc.vector.memset(kTz[0:64, :, 1, :], 0.0)
        k_aug = persist.tile([128, MB, H * (HD + 1)], bf16, tag="k_aug")
        lip_vT = persist.tile([128, CB, N], bf16, tag="lip_vT")
        qs = persist.tile([128, MB, C], bf16, tag="qs")        # qs natural [n, c]
        WprojN_sb = persist.tile([128, CB, C], bf16, tag="WprojN_sb")
        Wc_sb = persist.tile([128, CB, C], bf16, tag="Wc_sb")
        bprojP_sb = persist.tile([128, CB, HD + 1], bf16, tag="bprojP_sb")
        sel_sb = persist.tile([128, HD + 1], bf16, tag="sel_sb")
        zT = persist.tile([128, CB, N], bf16, tag="zT")
        av_attn = persist.tile([128, CB, C], bf16, tag="av_attn")
        # bias row for the output GEMM, as K=65 rhs (rows 1:64 zeroed so the
        # onesK zero-rows multiply clean values, never junk NaNs)
        row_bf = persist.tile([HD + 1, C], bf16, tag="row_bf")
        nc.gpsimd.memset(row_bf[:], 0.0)
        # K=65 broadcast stationaries (65 rounds to tile row size 128, so no
        # PE tiling-mode switches): selC row 0 -> cols 0:64, row 64 -> cols
        # 64:128; onesK row 0 -> all cols.  rhs tiles r2/s2 hold per-head
        # reciprocal rows at partitions 0 and 64, zeroed elsewhere; two
        # alternating copies avoid cross-p WAR stalls.
        selC = persist.tile([HD + 1, 128], bf16, tag="selC")
        onesK = persist.tile([HD + 1, 128], bf16, tag="onesK")
        r2s = [persist.tile([HD + 1, N], bf16, tag=f"r2_{i}", name=f"r2_{i}")
               for i in range(2)]
        s2s = [persist.tile([HD + 1, N], bf16, tag=f"s2_{i}", name=f"s2_{i}")
               for i in range(2)]
        for t in r2s + s2s:
            nc.gpsimd.memset(t[:], 0.0)

        attn = ctx.enter_context(tc.tile_pool(name="attn", bufs=1))
        early_ctx = tc.tile_pool(name="early", bufs=1)
        early = early_ctx.__enter__()
        audiaT_sb = early.tile([128, CB, N], bf16, tag="audiaT_sb")
        lipT_sb = early.tile([128, CB, N], bf16, tag="lipT_sb")
        WqkT_sb = early.tile([128, CB, 2 * C], bf16, tag="WqkT_sb")
        WlipT_sb = early.tile([128, CB, C], bf16, tag="WlipT_sb")
        WseT_sb = early.tile([128, CB, C], bf16, tag="WseT_sb")

        # input DMAs ordered by first use and spread across engine DMA rings:
        # qkT needs WqkT(q half) [sync ring] + audiaT [scalar ring] first;
        # the k half [vector ring] feeds emit_kaug; lip/se/proj follow.
        d_WqkT_r = d_WqkT[:].rearrange("(cb p) c -> cb p c", p=128)
        d_audiaT_r = d_audiaT[:].rearrange("(cb p) n -> cb p n", p=128)
        for cb in range(CB):
            nc.sync.dma_start(WqkT_sb[:, cb, 0:C], d_WqkT_r[cb][:, 0:C])
            nc.scalar.dma_start(audiaT_sb[:, cb], d_audiaT_r[cb])
        for cb in range(CB):
            nc.sync.dma_start(WqkT_sb[:, cb, C:2 * C], d_WqkT_r[cb][:, C:2 * C])
        nc.scalar.dma_start(WlipT_sb[:], d_WlipT[:].rearrange("(cb p) c -> p cb c", p=128))
        nc.scalar.dma_start(lipT_sb[:], d_lipT[:].rearrange("(cb p) n -> p cb n", p=128))
        nc.sync.dma_start(sel_sb[:], d_sel[:])
        nc.sync.dma_start(selC[:], d_selC[:])
        nc.sync.dma_start(onesK[:], d_onesK[:])
        nc.sync.dma_start(WseT_sb[:], d_WseT[:].rearrange("(cb p) c -> p cb c", p=128))
        nc.scalar.dma_start(WprojN_sb[:], d_WprojN[:].rearrange("(cb p) c -> p cb c", p=128))
        nc.scalar.dma_start(bprojP_sb[:], d_bprojP[:])

        # ---- P1 pieces (emitted interleaved with attention below) ----
        def emit_qkT():
            # qT / kTz: out [oc:128, n] ; lhsT = WqkT block, rhs = audiaT.
            # Interleave q and k per ocb so S(0)'s deps (ocb-0 chunks of both)
            # land first and the ACT exp stream starts as early as possible;
            # all evacs stay on the DVE to keep the ACT queue pure exps.
            for ocb in range(CB):
                for dst, oc0 in ((qT, 0), (kTz, C)):
                    ps = psA.tile([128, N], f32, tag="psA")
                    for cb in range(CB):
                        for nh in range(NH):
                            nc.tensor.matmul(
                                ps[:, nh * 512:(nh + 1) * 512],
                                WqkT_sb[:, cb, oc0 + ocb * 128: oc0 + (ocb + 1) * 128],
                                audiaT_sb[:, cb, nh * 512:(nh + 1) * 512],
                                start=(cb == 0), stop=(cb == CB - 1),
                            )
                    if dst is qT:
                        nc.vector.tensor_copy(out=dst[:, ocb, :], in_=ps[:])
                    else:
                        nc.vector.tensor_copy(out=kTz[0:64, ocb, 0, :], in_=ps[0:64, :])
                        nc.vector.tensor_copy(out=kTz[64:128, ocb, 1, :], in_=ps[64:128, :])

        def emit_kaug():
            # k natural (+ trailing ones column: raw row 64 = softmax denom r)
            for mb in range(MB):
                ps = psB.tile([128, N], f32, tag="psB")
                for cb in range(CB):
                    nc.tensor.matmul(
                        ps[:, 0:512],
                        audiaT_sb[:, cb, mb * 128:(mb + 1) * 128],
                        WqkT_sb[:, cb, C:2 * C],
                        start=(cb == 0), stop=(cb == CB - 1),
                    )
                nc.vector.tensor_copy(
                    out=k_aug[:, mb].rearrange("p (h e) -> p h e", e=HD + 1)[:, :, 0:HD],
                    in_=ps[:, 0:512].rearrange("p (h d) -> p h d", d=HD),
                )
            nc.vector.memset(
                k_aug[:].rearrange("p m (h e) -> p m h e", e=HD + 1)[:, :, :, HD:HD + 1], 1.0
            )

        def emit_lipv():
            for ocb in range(CB):
                ps = psB.tile([128, N], f32, tag="psB")
                for cb in range(CB):
                    for nh in range(NH):
                        nc.tensor.matmul(
                            ps[:, nh * 512:(nh + 1) * 512],
                            WlipT_sb[:, cb, ocb * 128:(ocb + 1) * 128],
                            lipT_sb[:, cb, nh * 512:(nh + 1) * 512],
                            start=(cb == 0), stop=(cb == CB - 1),
                        )
                nc.vector.tensor_copy(out=lip_vT[:, ocb, :], in_=ps[:])

        def emit_qs():
            # qs natural [n, c] (needed on both sides of the SE bmm)
            for nb in range(MB):
                ps = psB.tile([128, N], f32, tag="psB")
                for cb in range(CB):
                    nc.tensor.matmul(
                        ps[:, 0:512],
                        audiaT_sb[:, cb, nb * 128:(nb + 1) * 128],
                        WseT_sb[:, cb, :],
                        start=(cb == 0), stop=(cb == CB - 1),
                    )
                nc.vector.tensor_copy(out=qs[:, nb, :], in_=ps[:, 0:512])

        # ---- attention + SE ----
        def emit_se():
            # x[c,d] = sum_n qs[n,c] qs[n,d];  av_attn = sigmoid(2x/temp)
            for cb in range(CB):
                ps = psB.tile([128, N], f32, tag="psB")
                for nb in range(MB):
                    nc.tensor.matmul(
                        ps[:, 0:512],
                        qs[:, nb, cb * 128:(cb + 1) * 128],
                        qs[:, nb, :],
                        start=(nb == 0), stop=(nb == MB - 1),
                    )
                nc.scalar.activation(av_attn[:, cb, :], ps[:, 0:512], AF.Sigmoid,
                                     scale=2.0 / TEMP)

        def emit_wc():
            # Wc = W_proj.T @ av_attn  (fold the proj GEMM into the output GEMM)
            for ccb in range(CB):
                ps = psA.tile([128, N], f32, tag="psA")
                for eb in range(CB):
                    nc.tensor.matmul(
                        ps[:, 0:512],
                        WprojN_sb[:, eb, ccb * 128:(ccb + 1) * 128],
                        av_attn[:, eb, :],
                        start=(eb == 0), stop=(eb == CB - 1),
                    )
                nc.scalar.activation(Wc_sb[:, ccb, :], ps[:, 0:512], AF.Copy)
            # row = b_proj @ av_attn (M=65 zero-padded stationary: no tiling
            # switch); prefolded into the partial-output psum chains via a
            # K=65 onesK matmul.
            rp = psB.tile([128, N], f32, tag="psB")
            for cb in range(CB):
                nc.tensor.matmul(
                    rp[0:HD + 1, 0:512],
                    bprojP_sb[:, cb, :],
                    av_attn[:, cb, :],
                    start=(cb == 0), stop=(cb == CB - 1),
                )
            nc.vector.tensor_copy(out=row_bf[0:1, :], in_=rp[0:1, 0:512])
            if DEBUG_DUMP:
                nc.sync.dma_start(dbg["dbg_wc"][:], Wc_sb[:])

        def emit_S(p):
            # E = exp(S.T * scale) for heads (2p, 2p+1)
            E = attn.tile([128, MB, 2 * N], bf16, tag="E", bufs=2, name=f"E{p}")
            for mb in range(MB):
                for hh in range(2):
                    ps = psA.tile([128, N], f32, tag="psA")
                    for nh in range(NH):
                        nc.tensor.matmul(
                            ps[:, nh * 512:(nh + 1) * 512],
                            kTz[:, p, hh, mb * 128:(mb + 1) * 128],
                            qT[:, p, nh * 512:(nh + 1) * 512],
                            start=True, stop=True,
                        )
                    nc.scalar.activation(
                        E[:, mb, hh * N:(hh + 1) * N], ps[:], AF.Exp, scale=SCALE
                    )
            return E

        def emit_qk_front(p, E):
            # raw = [k_h|1].T @ E_h for BOTH heads back-to-back (keeps the PE
            # dense so HAM stays at full clock), then the first-softmax
            # normalize chain through y/expY.  The s-path + z live in
            # emit_qk_back so PE filler work can be emitted between them
            # (the PE queue is in-order: a stalled s_ps matmul would block
            # every filler emitted after it).
            attn_sb = attn.tile([128, N], bf16, tag="attn_sb", bufs=2, name=f"attn_sb{p}")
            raw2 = attn.tile([128, N], bf16, tag="raw_sb", bufs=2, name=f"raw2_{p}")
            r0s = []
            for hh in range(2):
                h = 2 * p + hh
                raw = psB.tile([128, N], f32, tag="psB")
                for mb in range(MB):
                    for nh in range(NH):
                        nc.tensor.matmul(
                            raw[0:HD + 1, nh * 512:(nh + 1) * 512],
                            k_aug[:, mb, h * (HD + 1):(h + 1) * (HD + 1)],
                            E[:, mb, hh * N + nh * 512: hh * N + (nh + 1) * 512],
                            start=(mb == 0), stop=(mb == MB - 1),
                        )
                # psum->sbuf copies: both heads' attn rows pack into one
                # [128,N] tile (partition-shifted write for the hi head).
                # 1/r via the ACT Reciprocal LUT straight from PSUM: the hi
                # head reads psum row 64 into r2 row 64 (aligned); the lo
                # head needs r on partition 0 first (DVE PSUM-read shift).
                # For p>=2 the ACT queue has drained its S exps, so the
                # partition-aligned lo-head ops ride ACT for free; anything
                # needing a partition shift stays on the DVE.
                if p >= 2 and hh == 0:
                    nc.scalar.activation(raw2[0:64, :], raw[0:HD, :], AF.Copy)
                else:
                    nc.vector.tensor_copy(
                        out=raw2[hh * 64:hh * 64 + 64, :], in_=raw[0:HD, :]
                    )
                r2 = r2s[p % 2]
                r0 = attn.tile([1, N], f32, tag="r_row", bufs=2, name=f"r0{p}_{hh}")
                nc.vector.tensor_copy(out=r0[:], in_=raw[HD:HD + 1, :])
                rinv = attn.tile([1, N], f32, tag="r_rec", bufs=2, name=f"rinv{p}_{hh}")
                nc.vector.reciprocal_approx_fast(out=rinv[:], in_=r0[:])
                if p >= 2 and hh == 0:
                    nc.scalar.activation(r2[0:1, :], rinv[:], AF.Copy)
                else:
                    nc.vector.tensor_copy(out=r2[hh * 64:hh * 64 + 1, :], in_=rinv[:])
                if hh == 0:
                    r0s.append(r0)
            # K=65 selector matmul broadcasts 1/r_lo to rows 0:64 and 1/r_hi
            # to rows 64:128 of one PSUM tile -> single normalize TT.  The
            # whole chain runs at nh-half granularity so downstream stages
            # (and ultimately the output finals) start on half-tiles early.
            rbps = (psA if p == 2 else psB).tile(
                [128, N], f32, tag="psA" if p == 2 else "psB"
            )
            y = attn.tile([128, N], bf16, tag="y", bufs=2, name=f"y{p}")
            expY = attn.tile([128, N], bf16, tag="expY", bufs=2, name=f"expY{p}")
            for nh in range(NH):
                sl = slice(nh * 512, (nh + 1) * 512)
                nc.tensor.matmul(rbps[:, sl], selC[:], r2[:, sl],
                                 start=True, stop=True)
                nc.vector.tensor_tensor(attn_sb[:, sl], raw2[:, sl], rbps[:, sl], MUL)
                nc.vector.tensor_tensor(y[:, sl], attn_sb[:, sl],
                                        lip_vT[:, p, sl], MUL)
                nc.scalar.activation(expY[:, sl], y[:, sl], AF.Exp)
            if DEBUG_DUMP and p == 0:
                nc.sync.dma_start(dbg["dbg_rawsb0"][:], raw2[0:HD, :])
                nc.sync.dma_start(dbg["dbg_r00"][:], r0s[0][:])
                nc.sync.dma_start(dbg["dbg_attn0"][:], attn_sb[:])
                nc.sync.dma_start(dbg["dbg_y0"][:], y[:])
                nc.sync.dma_start(dbg["dbg_expY0"][:], expY[:])
            return attn_sb, expY

        def emit_qk_back(p, attn_sb, expY):
            # s_lo lands at PSUM row 0, s_hi at row 64 (sel is [128,65] so
            # M=65 rounds to 128: no tiling-mode switch).  The whole s-path
            # runs at nh-half granularity so zT's first half lands early -
            # the output finals for nb 0..3 only need columns 0:512.
            # For p==2 park the psums on psA - they hold their buffers to
            # the end of the chain and would starve the psB ring the se/wc
            # fillers need.  (p==3 must stay on psB: psA holds the output
            # partials.)
            s_ps = psB.tile([128, N], f32, tag="psB")
            sbps = (psA if p == 2 else psB).tile(
                [128, N], f32, tag="psA" if p == 2 else "psB"
            )
            s2 = s2s[p % 2]
            u = attn.tile([128, N], bf16, tag="y", bufs=2, name=f"u{p}")
            for nh in range(NH):
                sl = slice(nh * 512, (nh + 1) * 512)
                nc.tensor.matmul(s_ps[0:HD + 1, sl], sel_sb[:], expY[:, sl],
                                 start=True, stop=True)
                # 1/s: the lo row is PSUM-partition-aligned (0->0) so its
                # copy and cast ride the ACT engine once it has drained its
                # S exps; the hi row needs the DVE's partition-shifting
                # PSUM read and shifted write.
                for hh in range(2):
                    sx = attn.tile([1, 512], f32, tag="r_row", bufs=2,
                                   name=f"sx{p}_{hh}_{nh}")
                    if p >= 2 and hh == 0:
                        nc.scalar.activation(sx[:], s_ps[0:1, sl], AF.Copy)
                    else:
                        nc.vector.tensor_copy(
                            out=sx[:], in_=s_ps[hh * 64:hh * 64 + 1, sl]
                        )
                    sxi = attn.tile([1, 512], f32, tag="r_rec", bufs=2,
                                    name=f"sxi{p}_{hh}_{nh}")
                    nc.vector.reciprocal_approx_fast(out=sxi[:], in_=sx[:])
                    if p >= 2 and hh == 0:
                        nc.scalar.activation(s2[0:1, sl], sxi[:], AF.Copy)
                    else:
                        nc.vector.tensor_copy(
                            out=s2[hh * 64:hh * 64 + 1, sl], in_=sxi[:]
                        )
                nc.tensor.matmul(sbps[:, sl], selC[:], s2[:, sl],
                                 start=True, stop=True)
                nc.vector.tensor_tensor(u[:, sl], attn_sb[:, sl], expY[:, sl], MUL)
                nc.vector.tensor_tensor(zT[:, p, sl], u[:, sl], sbps[:, sl], MUL)

        # 2-deep software pipeline: S(p+1) overlaps qk(p); the independent
        # projection/SE matmuls are spread through the ACT-bound S windows
        # as PE filler.  se/wc fill qk(2)'s normalize window; the bias +
        # cb=0..2 partial output accumulation (evacuated to SBUF bf16 by the
        # idle ACT engine) fills qk(3)'s; only the cb=3 matmul + one add +
        # DMA remain after zT(3) lands.
        emit_qkT()
        E0 = emit_S(0)
        emit_kaug()
        E1 = emit_S(1)
        emit_lipv()
        fb0 = emit_qk_front(0, E0)
        if DEBUG_DUMP:
            nc.sync.dma_start(dbg["dbg_E0"][:], E0[:])
        E2 = emit_S(2)
        emit_qk_back(0, *fb0)
        emit_qs()
        early_ctx.__exit__(None, None, None)
        fb1 = emit_qk_front(1, E1)
        E3 = emit_S(3)
        emit_qk_back(1, *fb1)
        fb2 = emit_qk_front(2, E2)
        emit_se()
        emit_wc()
        emit_qk_back(2, *fb2)
        fb3 = emit_qk_front(3, E3)
        # bias + cb=0..2 partial output accumulation for all 8 nb chunks;
        # two nb chunks pack into each [128,1024] psA tile, evacuated to
        # partial_sb so the psA ring keeps cycling.
        late = ctx.enter_context(tc.tile_pool(name="late", bufs=1))
        partial_sb = late.tile([128, MB, C], bf16, tag="partial_sb")

        def emit_partials(ilo, ihi):
            for i in range(ilo, ihi):
                ps = psA.tile([128, N], f32, tag="psA")
                for half in range(2):
                    nb = 2 * i + half
                    nc.tensor.matmul(
                        ps[:, half * 512:(half + 1) * 512],
                        onesK[:], row_bf[:],
                        start=True, stop=False,
                    )
                    for cb in range(CB - 1):
                        nc.tensor.matmul(
                            ps[:, half * 512:(half + 1) * 512],
                            zT[:, cb, nb * 128:(nb + 1) * 128],
                            Wc_sb[:, cb, :],
                            start=False, stop=(cb == CB - 2),
                        )
                nc.scalar.activation(partial_sb[:, 2 * i:2 * i + 2, :], ps[:], AF.Copy)

        # first half before back(3) so its s-path matmuls aren't queued
        # behind all 40 partial MMs; second half fills back(3)'s window
        emit_partials(0, 2)
        emit_qk_back(3, *fb3)
        emit_partials(2, 4)
        if DEBUG_DUMP:
            nc.sync.dma_start(dbg["dbg_qT"][:], qT[:])
            nc.sync.dma_start(dbg["dbg_kTz"][:], kTz[:])
            nc.sync.dma_start(dbg["dbg_kaug"][:], k_aug[:])
            nc.sync.dma_start(dbg["dbg_lipv"][:], lip_vT[:])
            nc.sync.dma_start(dbg["dbg_qs"][:], qs[:])
            nc.sync.dma_start(dbg["dbg_zT"][:], zT[:])
            nc.sync.dma_start(dbg["dbg_avattn"][:], av_attn[:])

        # ---- output finals: out[nb] = partial_sb[nb] + z[:,3] @ Wc[3] ----
        d_out_r = d_out[:].rearrange("(nb p) c -> nb p c", p=128)
        with tc.tile_pool(name="outp", bufs=4) as outp:
            for i in range(4):
                ps = psA.tile([128, N], f32, tag="psA")
                for half in range(2):
                    nb = 2 * i + half
                    nc.tensor.matmul(
                        ps[:, half * 512:(half + 1) * 512],
                        zT[:, CB - 1, nb * 128:(nb + 1) * 128],
                        Wc_sb[:, CB - 1, :],
                        start=True, stop=True,
                    )
                for half in range(2):
                    nb = 2 * i + half
                    o_sb = outp.tile([128, C], bf16, tag="o_sb")
                    nc.vector.tensor_tensor(
                        o_sb[:], ps[:, half * 512:(half + 1) * 512],
                        partial_sb[:, nb, :], ADD,
                    )
                    if nb % 2 == 0:
                        nc.scalar.dma_start(d_out_r[nb], o_sb[:])
                    else:
                        nc.sync.dma_start(d_out_r[nb], o_sb[:])

    nc.compile()
    return nc


def _marshal(audia, lip, W_qkv, W_lip, W_proj, b_proj, W_se):
    bf16 = ml_dtypes.bfloat16
    WqkT = np.ascontiguousarray(W_qkv[:2 * C].T.astype(bf16))
    WlipT = np.ascontiguousarray(W_lip.T.astype(bf16))
    WseT = np.ascontiguousarray(W_se.T.astype(bf16))
    WprojN = np.ascontiguousarray(W_proj.astype(bf16))
    bprojP = np.zeros((128, CB, HD + 1), bf16)
    bprojP[:, :, 0] = np.asarray(b_proj, np.float32).reshape(CB, 128).T.astype(bf16)
    sel = np.zeros((128, HD + 1), bf16)
    sel[0:64, 0] = 1
    sel[64:128, HD] = 1
    selC = np.zeros((HD + 1, 128), bf16)
    selC[0, 0:64] = 1
    selC[HD, 64:128] = 1
    onesK = np.zeros((HD + 1, 128), bf16)
    onesK[0, :] = 1
    in_maps = []
    for b in range(B):
        in_maps.append({
            "audiaT": np.ascontiguousarray(audia[b].T.astype(bf16)),
            "lipT": np.ascontiguousarray(lip[b].T.astype(bf16)),
            "WqkT": WqkT, "WlipT": WlipT, "WseT": WseT, "WprojN": WprojN,
            "bprojP": bprojP, "sel": sel, "selC": selC, "onesK": onesK,
        })
    return in_maps


def run(inputs, trace=False, **kw):
    from concourse.bass_utils import run_bass_kernel_spmd
    if "nc" not in _CACHED:
        _CACHED["nc"] = build_nc()
    in_maps = _marshal(**inputs)
    return run_bass_kernel_spmd(
        _CACHED["nc"], in_maps, core_ids=list(range(B)), trace=trace, **kw
    )


def kernel(audia, lip, W_qkv, W_lip, W_proj, b_proj, W_se):
    res = run(dict(audia=audia, lip=lip, W_qkv=W_qkv, W_lip=W_lip,
                   W_proj=W_proj, b_proj=b_proj, W_se=W_se))
    return np.stack([r["out"] for r in res.results], 0).astype(np.float32)


# revision 88
# speedup vs baseline: 1.0197x; 1.0197x over previous
"""Trainium2 Bass kernel for nn_AV_MiltiHeadAttention.

Strategy: data-parallel over B across 8 NeuronCores (1 batch element per core).
Per-core everything is kept in a "transposed" (feature-on-partitions) layout so
no on-chip transposes are needed:

  audiaT/lipT/W*T are marshalled on host (transpose + bf16 cast = input
  layout prep for the SPMD shards), all matmul contractions then have their
  contraction dim on SBUF partitions naturally.

  S.T[m,n] = sum_d kTz[d,m] qT[d,n]         (zero-padded per head: K=128, so
                                             every matmul shares one PE tiling
                                             mode -> no array-drain switches)
  E = exp(S.T * scale)                       (no max subtraction; |S*scale|<2)
  raw[j,n] = sum_m [k_h | 1][m,j] E[m,n]    -> rows 0..63 = attn_qk.T (unnorm),
                                               row 64 = softmax denominator r
  1/r, 1/s chains on [1,N] rows (DVE recip; partition-0 constraint), placed
  at rows 0/64 of zeroed [65,N] tiles; K=65 selector matmuls broadcast them
  across all 128 partitions in PSUM (65 rounds to row-size 128: no mode
  switch), so each normalize is a single [128,N] tensor_tensor.
  y = attn_qk.T * lip_v.T ; expY = exp(y)
  s = colsum_d(expY) via [128,65] selector matmul (softmax over head_dim)
  z.T = attn_qk.T * expY * (1/s)
  av_attn = sigmoid(2 * qs.T@qs / temp)      (SE layer)
  Wc = W_proj.T @ av_attn ; row = b_proj @ av_attn   (proj/out fusion)
  out = z @ Wc + row                          (single fused output GEMM; bias
                                             and cb<3 partials accumulate
                                             early, only cb=3 + add + DMA
                                             remain after the last z lands)

All matmuls bf16 x bf16 -> fp32 PSUM. Softmax normalization in fp32.
"""

import os
import numpy as np
import ml_dtypes

DEBUG_DUMP = os.environ.get("KDBG", "0") == "1"

B, N, C = 8, 1024, 512
H, HD = 8, 64
CB = C // 128          # 4 chunks of the feature dim
MB = N // 128          # 8 chunks of the token dim
NH = N // 512          # 2 halves of the token dim (matmul free dim = 512)
SCALE = HD ** -0.5
TEMP = C ** 0.5

_CACHED = {}


def build_nc():
    import concourse.bass as bass
    import concourse.tile as tile
    import concourse.mybir as mybir
    from concourse import bacc
    from contextlib import ExitStack

    f32 = mybir.dt.float32
    bf16 = mybir.dt.bfloat16
    AF = mybir.ActivationFunctionType
    MUL = mybir.AluOpType.mult
    ADD = mybir.AluOpType.add

    nc = bacc.Bacc("TRN2", target_bir_lowering=False, debug=False, num_devices=B)

    d_audiaT = nc.dram_tensor("audiaT", [C, N], bf16, kind="ExternalInput")
    d_lipT = nc.dram_tensor("lipT", [C, N], bf16, kind="ExternalInput")
    d_WqkT = nc.dram_tensor("WqkT", [C, 2 * C], bf16, kind="ExternalInput")
    d_WlipT = nc.dram_tensor("WlipT", [C, C], bf16, kind="ExternalInput")
    d_WseT = nc.dram_tensor("WseT", [C, C], bf16, kind="ExternalInput")
    d_WprojN = nc.dram_tensor("WprojN", [C, C], bf16, kind="ExternalInput")
    d_bprojP = nc.dram_tensor("bprojP", [128, CB, HD + 1], bf16, kind="ExternalInput")
    d_sel = nc.dram_tensor("sel", [128, HD + 1], bf16, kind="ExternalInput")
    d_selC = nc.dram_tensor("selC", [HD + 1, 128], bf16, kind="ExternalInput")
    d_onesK = nc.dram_tensor("onesK", [HD + 1, 128], bf16, kind="ExternalInput")
    d_out = nc.dram_tensor("out", [N, C], bf16, kind="ExternalOutput")
    dbg = {}
    if DEBUG_DUMP:
        for nm, shp, dt in [
            ("dbg_qT", [128, CB, N], bf16), ("dbg_kTz", [128, CB, 2, N], bf16),
            ("dbg_kaug", [128, MB, H * (HD + 1)], bf16),
            ("dbg_lipv", [128, CB, N], bf16), ("dbg_qs", [128, MB, C], bf16),
            ("dbg_E0", [128, MB, 2 * N], bf16),
            ("dbg_rawsb0", [HD, N], bf16), ("dbg_attn0", [128, N], bf16),
            ("dbg_r00", [1, N], f32),
            ("dbg_y0", [128, N], bf16), ("dbg_expY0", [128, N], bf16),
            ("dbg_sblo0", [HD, N], f32), ("dbg_sbhi0", [HD, N], f32),
            ("dbg_zT", [128, CB, N], bf16), ("dbg_avattn", [128, CB, C], bf16),
            ("dbg_wc", [128, CB, C], bf16),
        ]:
            dbg[nm] = nc.dram_tensor(nm, shp, dt, kind="ExternalOutput")

    with tile.TileContext(nc) as tc, ExitStack() as ctx:
        persist = ctx.enter_context(tc.tile_pool(name="persist", bufs=1))
        psA = ctx.enter_context(tc.tile_pool(name="psA", bufs=2, space="PSUM"))
        psB = ctx.enter_context(tc.tile_pool(name="psB", bufs=2, space="PSUM"))

        # ---- persistent SBUF tensors ----
        qT = persist.tile([128, CB, N], bf16, tag="qT")        # q.T  [(h,d), n]
        # k.T zero-padded per head so the S matmuls run K=128 (no PE
        # tiling-mode switches): chunk 0 = [k_h_lo; 0], chunk 1 = [0; k_h_hi]
        kTz = persist.tile([128, CB, 2, N], bf16, tag="kTz")
        nc.vector.memset(kTz[64:128, :, 0, :], 0.0)
        nc.vector.memset(kTz[0:64, :, 1, :], 0.0)
        k_aug = persist.tile([128, MB, H * (HD + 1)], bf16, tag="k_aug")
        lip_vT = persist.tile([128, CB, N], bf16, tag="lip_vT")
        qs = persist.tile([128, MB, C], bf16, tag="qs")        # qs natural [n, c]
        WprojN_sb = persist.tile([128, CB, C], bf16, tag="WprojN_sb")
        Wc_sb = persist.tile([128, CB, C], bf16, tag="Wc_sb")
        bprojP_sb = persist.tile([128, CB, HD + 1], bf16, tag="bprojP_sb")
        sel_sb = persist.tile([128, HD + 1], bf16, tag="sel_sb")
        zT = persist.tile([128, CB, N], bf16, tag="zT")
        av_attn = persist.tile([128, CB, C], bf16, tag="av_attn")
        # bias row for the output GEMM, as K=65 rhs (rows 1:64 zeroed so the
        # onesK zero-rows multiply clean values, never junk NaNs)
        row_bf = persist.tile([HD + 1, C], bf16, tag="row_bf")
        nc.gpsimd.memset(row_bf[:], 0.0)
        # K=65 broadcast stationaries (65 rounds to tile row size 128, so no
        # PE tiling-mode switches): selC row 0 -> cols 0:64, row 64 -> cols
        # 64:128; onesK row 0 -> all cols.  rhs tiles r2/s2 hold per-head
        # reciprocal rows at partitions 0 and 64, zeroed elsewhere; two
        # alternating copies avoid cross-p WAR stalls.
        selC = persist.tile([HD + 1, 128], bf16, tag="selC")
        onesK = persist.tile([HD + 1, 128], bf16, tag="onesK")
        r2s = [persist.tile([HD + 1, N], bf16, tag=f"r2_{i}", name=f"r2_{i}")
               for i in range(2)]
        s2s = [persist.tile([HD + 1, N], bf16, tag=f"s2_{i}", name=f"s2_{i}")
               for i in range(2)]
        for t in r2s + s2s:
            nc.gpsimd.memset(t[:], 0.0)

        attn = ctx.enter_context(tc.tile_pool(name="attn", bufs=1))
        early_ctx = tc.tile_pool(name="early", bufs=1)
        early = early_ctx.__enter__()
        audiaT_sb = early.tile([128, CB, N], bf16, tag="audiaT_sb")
        lipT_sb = early.tile([128, CB, N], bf16, tag="lipT_sb")
        WqkT_sb = early.tile([128, CB, 2 * C], bf16, tag="WqkT_sb")
        WlipT_sb = early.tile([128, CB, C], bf16, tag="WlipT_sb")
        WseT_sb = early.tile([128, CB, C], bf16, tag="WseT_sb")

        # input DMAs ordered by first use and spread across engine DMA rings:
        # qkT needs WqkT(q half) [sync ring] + audiaT [scalar ring] first;
        # the k half [vector ring] feeds emit_kaug; lip/se/proj follow.
        d_WqkT_r = d_WqkT[:].rearrange("(cb p) c -> cb p c", p=128)
        d_audiaT_r = d_audiaT[:].rearrange("(cb p) n -> cb p n", p=128)
        for cb in range(CB):
            nc.sync.dma_start(WqkT_sb[:, cb, 0:C], d_WqkT_r[cb][:, 0:C])
            nc.scalar.dma_start(audiaT_sb[:, cb], d_audiaT_r[cb])
        for cb in range(CB):
            nc.sync.dma_start(WqkT_sb[:, cb, C:2 * C], d_WqkT_r[cb][:, C:2 * C])
        nc.scalar.dma_start(WlipT_sb[:], d_WlipT[:].rearrange("(cb p) c -> p cb c", p=128))
        nc.scalar.dma_start(lipT_sb[:], d_lipT[:].rearrange("(cb p) n -> p cb n", p=128))
        nc.sync.dma_start(sel_sb[:], d_sel[:])
        nc.sync.dma_start(selC[:], d_selC[:])
        nc.sync.dma_start(onesK[:], d_onesK[:])
        nc.sync.dma_start(WseT_sb[:], d_WseT[:].rearrange("(cb p) c -> p cb c", p=128))
        nc.scalar.dma_start(WprojN_sb[:], d_WprojN[:].rearrange("(cb p) c -> p cb c", p=128))
        nc.scalar.dma_start(bprojP_sb[:], d_bprojP[:])

        # ---- P1 pieces (emitted interleaved with attention below) ----
        def emit_qkT():
            # qT / kTz: out [oc:128, n] ; lhsT = WqkT block, rhs = audiaT
            for dst, oc0 in ((qT, 0), (kTz, C)):
                for ocb in range(CB):
                    ps = psA.tile([128, N], f32, tag="psA")
                    for cb in range(CB):
                        for nh in range(NH):
                            nc.tensor.matmul(
                                ps[:, nh * 512:(nh + 1) * 512],
                                WqkT_sb[:, cb, oc0 + ocb * 128: oc0 + (ocb + 1) * 128],
                                audiaT_sb[:, cb, nh * 512:(nh + 1) * 512],
                                start=(cb == 0), stop=(cb == CB - 1),
                            )
                    if dst is qT:
                        nc.vector.tensor_copy(out=dst[:, ocb, :], in_=ps[:])
                    else:
                        nc.vector.tensor_copy(out=kTz[0:64, ocb, 0, :], in_=ps[0:64, :])
                        nc.scalar.activation(kTz[64:128, ocb, 1, :], ps[64:128, :], AF.Copy)

        def emit_kaug():
            # k natural (+ trailing ones column: raw row 64 = softmax denom r)
            for mb in range(MB):
                ps = psB.tile([128, N], f32, tag="psB")
                for cb in range(CB):
                    nc.tensor.matmul(
                        ps[:, 0:512],
                        audiaT_sb[:, cb, mb * 128:(mb + 1) * 128],
                        WqkT_sb[:, cb, C:2 * C],
                        start=(cb == 0), stop=(cb == CB - 1),
                    )
                nc.vector.tensor_copy(
                    out=k_aug[:, mb].rearrange("p (h e) -> p h e", e=HD + 1)[:, :, 0:HD],
                    in_=ps[:, 0:512].rearrange("p (h d) -> p h d", d=HD),
                )
            nc.vector.memset(
                k_aug[:].rearrange("p m (h e) -> p m h e", e=HD + 1)[:, :, :, HD:HD + 1], 1.0
            )

        def emit_lipv():
            for ocb in range(CB):
                ps = psB.tile([128, N], f32, tag="psB")
                for cb in range(CB):
                    for nh in range(NH):
                        nc.tensor.matmul(
                            ps[:, nh * 512:(nh + 1) * 512],
                            WlipT_sb[:, cb, ocb * 128:(ocb + 1) * 128],
                            lipT_sb[:, cb, nh * 512:(nh + 1) * 512],
                            start=(cb == 0), stop=(cb == CB - 1),
                        )
                nc.vector.tensor_copy(out=lip_vT[:, ocb, :], in_=ps[:])

        def emit_qs():
            # qs natural [n, c] (needed on both sides of the SE bmm)
            for nb in range(MB):
                ps = psB.tile([128, N], f32, tag="psB")
                for cb in range(CB):
                    nc.tensor.matmul(
                        ps[:, 0:512],
                        audiaT_sb[:, cb, nb * 128:(nb + 1) * 128],
                        WseT_sb[:, cb, :],
                        start=(cb == 0), stop=(cb == CB - 1),
                    )
                nc.vector.tensor_copy(out=qs[:, nb, :], in_=ps[:, 0:512])

        # ---- attention + SE ----
        def emit_se():
            # x[c,d] = sum_n qs[n,c] qs[n,d];  av_attn = sigmoid(2x/temp)
            for cb in range(CB):
                ps = psB.tile([128, N], f32, tag="psB")
                for nb in range(MB):
                    nc.tensor.matmul(
                        ps[:, 0:512],
                        qs[:, nb, cb * 128:(cb + 1) * 128],
                        qs[:, nb, :],
                        start=(nb == 0), stop=(nb == MB - 1),
                    )
                nc.scalar.activation(av_attn[:, cb, :], ps[:, 0:512], AF.Sigmoid,
                                     scale=2.0 / TEMP)

        def emit_wc():
            # Wc = W_proj.T @ av_attn  (fold the proj GEMM into the output GEMM)
            for ccb in range(CB):
                ps = psA.tile([128, N], f32, tag="psA")
                for eb in range(CB):
                    nc.tensor.matmul(
                        ps[:, 0:512],
                        WprojN_sb[:, eb, ccb * 128:(ccb + 1) * 128],
                        av_attn[:, eb, :],
                        start=(eb == 0), stop=(eb == CB - 1),
                    )
                nc.scalar.activation(Wc_sb[:, ccb, :], ps[:, 0:512], AF.Copy)
            # row = b_proj @ av_attn (M=65 zero-padded stationary: no tiling
            # switch); prefolded into the partial-output psum chains via a
            # K=65 onesK matmul.
            rp = psB.tile([128, N], f32, tag="psB")
            for cb in range(CB):
                nc.tensor.matmul(
                    rp[0:HD + 1, 0:512],
                    bprojP_sb[:, cb, :],
                    av_attn[:, cb, :],
                    start=(cb == 0), stop=(cb == CB - 1),
                )
            nc.vector.tensor_copy(out=row_bf[0:1, :], in_=rp[0:1, 0:512])
            if DEBUG_DUMP:
                nc.sync.dma_start(dbg["dbg_wc"][:], Wc_sb[:])

        def emit_S(p):
            # E = exp(S.T * scale) for heads (2p, 2p+1)
            E = attn.tile([128, MB, 2 * N], bf16, tag="E", bufs=2, name=f"E{p}")
            for mb in range(MB):
                for hh in range(2):
                    ps = psA.tile([128, N], f32, tag="psA")
                    for nh in range(NH):
                        nc.tensor.matmul(
                            ps[:, nh * 512:(nh + 1) * 512],
                            kTz[:, p, hh, mb * 128:(mb + 1) * 128],
                            qT[:, p, nh * 512:(nh + 1) * 512],
                            start=True, stop=True,
                        )
                    nc.scalar.activation(
                        E[:, mb, hh * N:(hh + 1) * N], ps[:], AF.Exp, scale=SCALE
                    )
            return E

        def emit_qk_front(p, E):
            # raw = [k_h|1].T @ E_h for BOTH heads back-to-back (keeps the PE
            # dense so HAM stays at full clock), then the first-softmax
            # normalize chain through y/expY.  The s-path + z live in
            # emit_qk_back so PE filler work can be emitted between them
            # (the PE queue is in-order: a stalled s_ps matmul would block
            # every filler emitted after it).
            attn_sb = attn.tile([128, N], bf16, tag="attn_sb", bufs=2, name=f"attn_sb{p}")
            raw2 = attn.tile([128, N], bf16, tag="raw_sb", bufs=2, name=f"raw2_{p}")
            r0s = []
            for hh in range(2):
                h = 2 * p + hh
                raw = psB.tile([128, N], f32, tag="psB")
                for mb in range(MB):
                    for nh in range(NH):
                        nc.tensor.matmul(
                            raw[0:HD + 1, nh * 512:(nh + 1) * 512],
                            k_aug[:, mb, h * (HD + 1):(h + 1) * (HD + 1)],
                            E[:, mb, hh * N + nh * 512: hh * N + (nh + 1) * 512],
                            start=(mb == 0), stop=(mb == MB - 1),
                        )
                # psum->sbuf copies: both heads' attn rows pack into one
                # [128,N] tile (partition-shifted write for the hi head).
                # 1/r via the ACT Reciprocal LUT straight from PSUM: the hi
                # head reads psum row 64 into r2 row 64 (aligned); the lo
                # head needs r on partition 0 first (DVE PSUM-read shift).
                # For p>=2 the ACT queue has drained its S exps, so the
                # partition-aligned lo-head ops ride ACT for free; anything
                # needing a partition shift stays on the DVE.
                if p >= 2 and hh == 0:
                    nc.scalar.activation(raw2[0:64, :], raw[0:HD, :], AF.Copy)
                else:
                    nc.vector.tensor_copy(
                        out=raw2[hh * 64:hh * 64 + 64, :], in_=raw[0:HD, :]
                    )
                r2 = r2s[p % 2]
                r0 = attn.tile([1, N], f32, tag="r_row", bufs=2, name=f"r0{p}_{hh}")
                nc.vector.tensor_copy(out=r0[:], in_=raw[HD:HD + 1, :])
                rinv = attn.tile([1, N], f32, tag="r_rec", bufs=2, name=f"rinv{p}_{hh}")
                nc.vector.reciprocal_approx_fast(out=rinv[:], in_=r0[:])
                if p >= 2 and hh == 0:
                    nc.scalar.activation(r2[0:1, :], rinv[:], AF.Copy)
                else:
                    nc.vector.tensor_copy(out=r2[hh * 64:hh * 64 + 1, :], in_=rinv[:])
                if hh == 0:
                    r0s.append(r0)
            # K=65 selector matmul broadcasts 1/r_lo to rows 0:64 and 1/r_hi
            # to rows 64:128 of one PSUM tile -> single normalize TT.  The
            # whole chain runs at nh-half granularity so downstream stages
            # (and ultimately the output finals) start on half-tiles early.
            rbps = (psA if p == 2 else psB).tile(
                [128, N], f32, tag="psA" if p == 2 else "psB"
            )
            y = attn.tile([128, N], bf16, tag="y", bufs=2, name=f"y{p}")
            expY = attn.tile([128, N], bf16, tag="expY", bufs=2, name=f"expY{p}")
            for nh in range(NH):
                sl = slice(nh * 512, (nh + 1) * 512)
                nc.tensor.matmul(rbps[:, sl], selC[:], r2[:, sl],
                                 start=True, stop=True)
                nc.vector.tensor_tensor(attn_sb[:, sl], raw2[:, sl], rbps[:, sl], MUL)
                nc.vector.tensor_tensor(y[:, sl], attn_sb[:, sl],
                                        lip_vT[:, p, sl], MUL)
                nc.scalar.activation(expY[:, sl], y[:, sl], AF.Exp)
            if DEBUG_DUMP and p == 0:
                nc.sync.dma_start(dbg["dbg_rawsb0"][:], raw2[0:HD, :])
                nc.sync.dma_start(dbg["dbg_r00"][:], r0s[0][:])
                nc.sync.dma_start(dbg["dbg_attn0"][:], attn_sb[:])
                nc.sync.dma_start(dbg["dbg_y0"][:], y[:])
                nc.sync.dma_start(dbg["dbg_expY0"][:], expY[:])
            return attn_sb, expY

        def emit_qk_back(p, attn_sb, expY):
            # s_lo lands at PSUM row 0, s_hi at row 64 (sel is [128,65] so
            # M=65 rounds to 128: no tiling-mode switch).  The whole s-path
            # runs at nh-half granularity so zT's first half lands early -
            # the output finals for nb 0..3 only need columns 0:512.
            # For p==2 park the psums on psA - they hold their buffers to
            # the end of the chain and would starve the psB ring the se/wc
            # fillers need.  (p==3 must stay on psB: psA holds the output
            # partials.)
            s_ps = psB.tile([128, N], f32, tag="psB")
            sbps = (psA if p == 2 else psB).tile(
                [128, N], f32, tag="psA" if p == 2 else "psB"
            )
            s2 = s2s[p % 2]
            u = attn.tile([128, N], bf16, tag="y", bufs=2, name=f"u{p}")
            for nh in range(NH):
                sl = slice(nh * 512, (nh + 1) * 512)
                nc.tensor.matmul(s_ps[0:HD + 1, sl], sel_sb[:], expY[:, sl],
                                 start=True, stop=True)
                # 1/s: the lo row is PSUM-partition-aligned (0->0) so its
                # copy and cast ride the ACT engine once it has drained its
                # S exps; the hi row needs the DVE's partition-shifting
                # PSUM read and shifted write.
                for hh in range(2):
                    sx = attn.tile([1, 512], f32, tag="r_row", bufs=2,
                                   name=f"sx{p}_{hh}_{nh}")
                    if p >= 2 and hh == 0:
                        nc.scalar.activation(sx[:], s_ps[0:1, sl], AF.Copy)
                    else:
                        nc.vector.tensor_copy(
                            out=sx[:], in_=s_ps[hh * 64:hh * 64 + 1, sl]
                        )
                    sxi = attn.tile([1, 512], f32, tag="r_rec", bufs=2,
                                    name=f"sxi{p}_{hh}_{nh}")
                    nc.vector.reciprocal_approx_fast(out=sxi[:], in_=sx[:])
                    if p >= 2 and hh == 0:
                        nc.scalar.activation(s2[0:1, sl], sxi[:], AF.Copy)
                    else:
                        nc.vector.tensor_copy(
                            out=s2[hh * 64:hh * 64 + 1, sl], in_=sxi[:]
                        )
                nc.tensor.matmul(sbps[:, sl], selC[:], s2[:, sl],
                                 start=True, stop=True)
                nc.vector.tensor_tensor(u[:, sl], attn_sb[:, sl], expY[:, sl], MUL)
                nc.vector.tensor_tensor(zT[:, p, sl], u[:, sl], sbps[:, sl], MUL)

        # 2-deep software pipeline: S(p+1) overlaps qk(p); the independent
        # projection/SE matmuls are spread through the ACT-bound S windows
        # as PE filler.  se/wc fill qk(2)'s normalize window; the bias +
        # cb=0..2 partial output accumulation (evacuated to SBUF bf16 by the
        # idle ACT engine) fills qk(3)'s; only the cb=3 matmul + one add +
        # DMA remain after zT(3) lands.
        emit_qkT()
        E0 = emit_S(0)
        emit_kaug()
        E1 = emit_S(1)
        emit_lipv()
        fb0 = emit_qk_front(0, E0)
        if DEBUG_DUMP:
            nc.sync.dma_start(dbg["dbg_E0"][:], E0[:])
        E2 = emit_S(2)
        emit_qk_back(0, *fb0)
        emit_qs()
        early_ctx.__exit__(None, None, None)
        fb1 = emit_qk_front(1, E1)
        E3 = emit_S(3)
        emit_qk_back(1, *fb1)
        fb2 = emit_qk_front(2, E2)
        emit_se()
        emit_wc()
        emit_qk_back(2, *fb2)
        fb3 = emit_qk_front(3, E3)
        # bias + cb=0..2 partial output accumulation for all 8 nb chunks;
        # two nb chunks pack into each [128,1024] psA tile, evacuated to
        # partial_sb so the psA ring keeps cycling.
        late = ctx.enter_context(tc.tile_pool(name="late", bufs=1))
        partial_sb = late.tile([128, MB, C], bf16, tag="partial_sb")

        def emit_partials(ilo, ihi):
            for i in range(ilo, ihi):
                ps = psA.tile([128, N], f32, tag="psA")
                for half in range(2):
                    nb = 2 * i + half
                    nc.tensor.matmul(
                        ps[:, half * 512:(half + 1) * 512],
                        onesK[:], row_bf[:],
                        start=True, stop=False,
                    )
                    for cb in range(CB - 1):
                        nc.tensor.matmul(
                            ps[:, half * 512:(half + 1) * 512],
                            zT[:, cb, nb * 128:(nb + 1) * 128],
                            Wc_sb[:, cb, :],
                            start=False, stop=(cb == CB - 2),
                        )
                nc.scalar.activation(partial_sb[:, 2 * i:2 * i + 2, :], ps[:], AF.Copy)

        # first half before back(3) so its s-path matmuls aren't queued
        # behind all 40 partial MMs; second half fills back(3)'s window
        emit_partials(0, 2)
        emit_qk_back(3, *fb3)
        emit_partials(2, 4)
        if DEBUG_DUMP:
            nc.sync.dma_start(dbg["dbg_qT"][:], qT[:])
            nc.sync.dma_start(dbg["dbg_kTz"][:], kTz[:])
            nc.sync.dma_start(dbg["dbg_kaug"][:], k_aug[:])
            nc.sync.dma_start(dbg["dbg_lipv"][:], lip_vT[:])
            nc.sync.dma_start(dbg["dbg_qs"][:], qs[:])
            nc.sync.dma_start(dbg["dbg_zT"][:], zT[:])
            nc.sync.dma_start(dbg["dbg_avattn"][:], av_attn[:])

        # ---- output finals: out[nb] = partial_sb[nb] + z[:,3] @ Wc[3] ----
        d_out_r = d_out[:].rearrange("(nb p) c -> nb p c", p=128)
        with tc.tile_pool(name="outp", bufs=4) as outp:
            for i in range(4):
                ps = psA.tile([128, N], f32, tag="psA")
                for half in range(2):
                    nb = 2 * i + half
                    nc.tensor.matmul(
                        ps[:, half * 512:(half + 1) * 512],
                        zT[:, CB - 1, nb * 128:(nb + 1) * 128],
                        Wc_sb[:, CB - 1, :],
                        start=True, stop=True,
                    )
                for half in range(2):
                    nb = 2 * i + half
                    o_sb = outp.tile([128, C], bf16, tag="o_sb")
                    nc.vector.tensor_tensor(
                        o_sb[:], ps[:, half * 512:(half + 1) * 512],
                        partial_sb[:, nb, :], ADD,
                    )
                    if nb % 2 == 0:
                        nc.scalar.dma_start(d_out_r[nb], o_sb[:])
                    else:
                        nc.sync.dma_start(d_out_r[nb], o_sb[:])

    nc.compile()
    return nc


def _marshal(audia, lip, W_qkv, W_lip, W_proj, b_proj, W_se):
    bf16 = ml_dtypes.bfloat16
    WqkT = np.ascontiguousarray(W_qkv[:2 * C].T.astype(bf16))
    WlipT = np.ascontiguousarray(W_lip.T.astype(bf16))
    WseT = np.ascontiguousarray(W_se.T.astype(bf16))
    WprojN = np.ascontiguousarray(W_proj.astype(bf16))
    bprojP = np.zeros((128, CB, HD + 1), bf16)
    bprojP[:, :, 0] = np.asarray(b_proj, np.float32).reshape(CB, 128).T.astype(bf16)
    sel = np.zeros((128, HD + 1), bf16)
    sel[0:64, 0] = 1
    sel[64:128, HD] = 1
    selC = np.zeros((HD + 1, 128), bf16)
    selC[0, 0:64] = 1
    selC[HD, 64:128] = 1
    onesK = np.zeros((HD + 1, 128), bf16)
    onesK[0, :] = 1
    in_maps = []
    for b in range(B):
        in_maps.append({
            "audiaT": np.ascontiguousarray(audia[b].T.astype(bf16)),
            "lipT": np.ascontiguousarray(lip[b].T.astype(bf16)),
            "WqkT": WqkT, "WlipT": WlipT, "WseT": WseT, "WprojN": WprojN,
            "bprojP": bprojP, "sel": sel, "selC": selC, "onesK": onesK,
        })
    return in_maps


def run(inputs, trace=False, **kw):
    from concourse.bass_utils import run_bass_kernel_spmd
    if "nc" not in _CACHED:
        _CACHED["nc"] = build_nc()
    in_maps = _marshal(**inputs)
    return run_bass_kernel_spmd(
        _CACHED["nc"], in_maps, core_ids=list(range(B)), trace=trace, **kw
    )


def kernel(audia, lip, W_qkv, W_lip, W_proj, b_proj, W_se):
    res = run(dict(audia=audia, lip=lip, W_qkv=W_qkv, W_lip=W_lip,
                   W_proj=W_proj, b_proj=b_proj, W_se=W_se))
    return np.stack([r["out"] for r in res.results], 0).astype(np.float32)
